# revision 22
# baseline (speedup 1.0000x reference)
"""Trainium2 Bass kernel for nn_AggregateLayer (gnn_message_passing).

Strategy (8 NeuronCores, dst-node sharding):
  - Host: route/sort edges by (core, dst-tile), pad to uniform chunk counts,
    build int16 gather-index wraps and per-edge scalar arrays (pure layout).
    x is pre-cast to bf16 (halves the edge-gather HBM traffic).
  - Phase 1 (per core, 2500 dst nodes): per relation, dma_gather bf16 x rows
    for each 128-edge chunk, build the bf16 scatter matrix
    S[e, dstlocal] = coef_e via iota/is_equal/mult on DVE, and accumulate
    PSUM[dst, :] += S^T @ G on the PE.  Denominators via per-dst padded
    coefficient rows + free-dim reduce (bf16-rounded exp to match the
    numerator's rounding).
  - AllGather bf16 H shards (5MB/core) into a Shared DRAM tensor, split in
    row chunks overlapped with phase 1.
  - Phase 2: two-stage software pipeline per 128-node tile: stage A gathers
    the K=16 candidate bf16 H rows, computes diff/squares/dist and the
    softmax att; stage B (emitted one tile later) does the att-weighted
    squared-diff mask and the final masked relation sum.  All bulk ops in
    bf16 for 2x DVE throughput.
"""

import numpy as np
import ml_dtypes

import concourse.bacc as bacc
from concourse.bass import AP
import concourse.mybir as mybir
import concourse.tile as tile
from concourse.bass_utils import run_bass_kernel_spmd
from concourse.library_config import mlp
from bass_rust import InstNoOp

F32 = mybir.dt.float32
BF16 = mybir.dt.bfloat16
I16 = mybir.dt.int16
AF = mybir.ActivationFunctionType
OP = mybir.AluOpType

R, NSRC, NVUL, D, E, K = 4, 20000, 20000, 256, 640000, 16
NCORES = 8
NSH = NVUL // NCORES          # 2500 dst nodes per core
TILES = (NSH + 127) // 128    # 20 tiles (last has 68 valid rows)
HROW = R * D                  # 1024 els per H row

# knobs
AG_COUNT = 1                  # timing instrument: emit AllGather this many times
AG_CHUNKS = 7                 # >1: split AllGather into row-chunks overlapped with phase 1
AG_SHARED = True              # AllGather into a Shared DRAM tensor (single copy)
DMA_SCRATCH = 32768           # SWDGE ring bytes
EMIT_REP = 1                  # repeat whole compute pass (timing instrument)
PHASES = "both"               # timing instrument: "both" | "p1" (skip phase2)
P1_MODE = "full"              # timing instrument: "full" | "nogather" | "gatheronly"
NQUEUES = 4                   # SWDGE queues; desc-gen parallelism across Q7 pairs
PIECE_CHUNKS = 6              # P1 gather piece size (chunks); ring holds 2048 descs
AG_MODE = "collective"        # "collective" payload AllGather | "direct" writes+barrier
VAR_CPT = False               # per-(tile,rel) chunk counts (max over cores) baked at compile
P2_SPLIT = True               # split phase-2 gather into two queue-spread pieces

_compiled = {}


# ---------------------------------------------------------------- host prep
def _wrap16(a):
    """dma_gather index layout: element i -> [i % 16, i // 16], tiled to 128
    partitions (8 Q7-core replicas)."""
    a = np.asarray(a, np.int16)
    pad = (-len(a)) % 16
    if pad:
        a = np.concatenate([a, np.zeros(pad, np.int16)])
    m = a.reshape(-1, 16).T
    return np.tile(m, (8, 1))


def _chunkify(v, cpt, fill):
    """[20, cpt*128] padded per-tile edge values -> [128, 20*cpt] chunk-major
    layout (edge t*cpt*128 + j*128 + p -> [p, t*cpt + j])."""
    out = v.reshape(TILES, cpt, 128).transpose(2, 0, 1).reshape(128, TILES * cpt)
    return np.ascontiguousarray(out)


def _host_prep(x_src, d, d1, d2, src_idx, dst_idx, cand_idx, splitvulid):
    split = int(splitvulid)
    x_src = np.asarray(x_src, np.float32)
    d = np.asarray(d, np.float32)
    d1 = np.asarray(d1, np.float32)
    d2 = np.asarray(d2, np.float32)
    src_idx = np.asarray(src_idx)
    dst_idx = np.asarray(dst_idx)
    cand_idx = np.asarray(cand_idx)

    # sort each relation's edges by dst once; split per core by searchsorted
    per_r = []
    for r in range(R):
        order = np.argsort(dst_idx[r], kind="stable")
        ds = dst_idx[r][order]
        ss = src_idx[r][order]
        bounds = np.searchsorted(ds, np.arange(0, NVUL + 1, NSH))
        per_r.append((ds, ss, bounds))

    # global uniform chunk count per dst-tile and max degree
    max_tile_edges = 0
    max_deg = 0
    tile_max = np.zeros((R, TILES), np.int64)   # per-(rel,tile) max edges over cores
    for r in range(R):
        ds, ss, bounds = per_r[r]
        for c in range(NCORES):
            dloc = ds[bounds[c]:bounds[c + 1]] - c * NSH
            tc_counts = np.bincount(dloc // 128, minlength=TILES)
            np.maximum(tile_max[r], tc_counts, out=tile_max[r])
            max_tile_edges = max(max_tile_edges, int(tc_counts.max()))
            deg = np.bincount(dloc, minlength=NSH)
            max_deg = max(max_deg, int(deg.max()))
    CPT = -(-max_tile_edges // 128)          # chunks per dst tile
    CPT += -CPT % 2                          # round to even (compile-cache)
    DMAX = max_deg + (-max_deg % 8)
    NCH = TILES * CPT
    CPTS = tuple(tuple(max(1, -(-int(tile_max[r][t]) // 128)) for r in range(R))
                 for t in range(TILES))

    maps = []
    for c in range(NCORES):
        m = {}
        for r in range(R):
            ds, ss, bounds = per_r[r]
            sl = slice(bounds[c], bounds[c + 1])
            dloc = ds[sl] - c * NSH
            sloc = ss[sl]
            dglob = ds[sl]
            nume = len(dloc)

            # per-edge scalars: dnum = d1[src] (dst<split) else -d2[src]
            use1 = dglob < split
            dnum = np.where(use1, d1[r][sloc], -d2[r][sloc]).astype(np.float32)
            dden = d[r][sloc].astype(np.float32)

            # scatter edges into per-tile padded slots [20, CPT*128]
            tid = dloc // 128
            starts = np.zeros(TILES, np.int64)
            cnt = np.bincount(tid, minlength=TILES)
            starts[1:] = np.cumsum(cnt)[:-1]
            pos = np.arange(nume) - starts[tid]     # position within tile
            slot = tid * (CPT * 128) + pos

            src_pad = np.zeros(TILES * CPT * 128, np.int16)
            dl_pad = np.full(TILES * CPT * 128, 200.0, np.float32)
            dn_pad = np.full(TILES * CPT * 128, -1e30, np.float32)
            dd_pad = np.ones(TILES * CPT * 128, np.float32)
            src_pad[slot] = sloc.astype(np.int16)
            dl_pad[slot] = (dloc % 128).astype(np.float32)
            dn_pad[slot] = dnum
            dd_pad[slot] = dden

            m[f"srcidx{r}"] = _wrap16(src_pad)
            m[f"dstloc{r}"] = _chunkify(dl_pad, CPT, 200.0)
            m[f"dnum{r}"] = _chunkify(dn_pad, CPT, -1e30)
            m[f"dden{r}"] = _chunkify(dd_pad, CPT, 1.0)

            # per-dst padded coefficient rows for the denominators
            deg = np.bincount(dloc, minlength=NSH)
            dstart = np.zeros(NSH, np.int64)
            dstart[1:] = np.cumsum(deg)[:-1]
            dpos = np.arange(nume) - dstart[dloc]
            cn = np.full((TILES * 128, DMAX), -1e30, np.float32)
            cd = np.ones((TILES * 128, DMAX), np.float32)
            cn[dloc, dpos] = dnum
            cd[dloc, dpos] = dden
            m[f"cpn{r}"] = np.ascontiguousarray(
                cn.reshape(TILES, 128, DMAX).transpose(1, 0, 2).reshape(128, TILES * DMAX))
            m[f"cpd{r}"] = np.ascontiguousarray(
                cd.reshape(TILES, 128, DMAX).transpose(1, 0, 2).reshape(128, TILES * DMAX))
            m[f"x{r}"] = np.ascontiguousarray(x_src[r].astype(ml_dtypes.bfloat16))

        # phase-2 candidate indices, per tile wrap (remapped to the chunked
        # hfull layout when the exchange is split into row-chunk AllGathers)
        if AG_CHUNKS > 1 and AG_MODE == "collective":
            rows_per = -(-TILES // AG_CHUNKS) * 128        # rows per chunk (tile-aligned)
            def remap(n):
                cc, loc = n // NSH, n % NSH
                q = np.minimum(loc // rows_per, AG_CHUNKS - 1)
                sz = np.minimum(NSH - q * rows_per, rows_per)
                base = NCORES * rows_per * q
                return base + cc * sz + (loc - q * rows_per)
        else:
            remap = lambda n: n
        ci = np.zeros((TILES, K * 128), np.int64)
        for t in range(TILES):
            base = c * NSH + t * 128
            nv = min(128, NSH - t * 128)
            blk = np.zeros((K, 128), np.int64)
            blk[:, :nv] = remap(cand_idx[base:base + nv, :].astype(np.int64)).T
            ci[t] = blk.reshape(-1)
        wr = np.concatenate([_wrap16(ci[t]) for t in range(TILES)], axis=1)
        m["candidx"] = wr
        maps.append(m)
    return maps, CPT, DMAX, CPTS


# ---------------------------------------------------------------- device build
def _fix_multiwaits(nc, limit=1):
    """This walrus build rejects >1-2 sem waits on one instruction; hoist
    excess waits onto same-engine NOPs inserted just before."""
    ctr = 0
    for bb in nc.m.functions[0].blocks:
        insts = bb.instructions
        out = []
        for inst in insts:
            si = inst.sync_info
            waits = list(si.on_wait) if (si and si.on_wait) else []
            if len(waits) > limit:
                excess, keep = waits[:-limit], waits[-limit:]
                for i in range(0, len(excess), limit):
                    ctr += 1
                    n = InstNoOp(name=f"I-mwfix-{ctr}", hint="mwfix")
                    n.engine = inst.engine
                    n.sync_info = mybir.SyncInfo(
                        on_wait=excess[i:i + limit], on_update=[])
                    out.append(n)
                si.on_wait = keep
            out.append(inst)
        if len(out) != len(insts):
            insts[:] = out
    return nc


def _build(CPT, DMAX, CPTS):
    NCH = TILES * CPT
    nc = bacc.Bacc("TRN2", target_bir_lowering=False, debug=False,
                   dynamic_dma_scratch_size=DMA_SCRATCH,
                   num_swdge_queues=NQUEUES)

    xs = [nc.declare_dram_parameter(f"x{r}", [NSRC, D], BF16, isOutput=False)
          for r in range(R)]
    srcidx = [nc.declare_dram_parameter(f"srcidx{r}", [128, NCH * 8], I16, isOutput=False)
              for r in range(R)]
    dstloc = [nc.declare_dram_parameter(f"dstloc{r}", [128, NCH], F32, isOutput=False)
              for r in range(R)]
    dnum = [nc.declare_dram_parameter(f"dnum{r}", [128, NCH], F32, isOutput=False)
            for r in range(R)]
    dden = [nc.declare_dram_parameter(f"dden{r}", [128, NCH], F32, isOutput=False)
            for r in range(R)]
    cpn = [nc.declare_dram_parameter(f"cpn{r}", [128, TILES * DMAX], F32, isOutput=False)
           for r in range(R)]
    cpd = [nc.declare_dram_parameter(f"cpd{r}", [128, TILES * DMAX], F32, isOutput=False)
           for r in range(R)]
    candidx = nc.declare_dram_parameter("candidx", [128, TILES * K * 8], I16, isOutput=False)
    out = nc.declare_dram_parameter("out", [NSH, D], F32, isOutput=True)

    hsh = nc.dram_tensor("hsh", [NSH, HROW], BF16)
    hfull = nc.dram_tensor("hfull", [NVUL, HROW], BF16,
                           addr_space="Shared" if AG_SHARED else "Local")

    with tile.TileContext(nc) as tc:
        with tc.tile_pool(name="const", bufs=1) as constp:
            nc.gpsimd.load_library(mlp)
            iota_i = constp.tile([128, 128], mybir.dt.int32)
            nc.gpsimd.iota(iota_i[:], pattern=[[1, 128]], base=0, channel_multiplier=0)
            iota_f = constp.tile([128, 128], F32)
            nc.vector.tensor_copy(iota_f[:], iota_i[:])
            iota_b = constp.tile([128, 128], BF16)
            nc.vector.tensor_copy(iota_b[:], iota_f[:])

            for rep in range(EMIT_REP):
                _emit_pass(nc, tc, iota_b, xs, srcidx, dstloc, dnum, dden,
                           cpn, cpd, candidx, out, hsh, hfull, CPT, DMAX, CPTS)

    _fix_multiwaits(nc)
    nc.compile()
    return nc


def _emit_pass(nc, tc, iota_b, xs, srcidx, dstloc, dnum, dden, cpn, cpd,
               candidx, out, hsh, hfull, CPT, DMAX, CPTS):
    NCH = TILES * CPT

    # ---------------- phase 1 ----------------
    if PHASES != "p2":
      with tc.tile_pool(name="p1res", bufs=1) as resp, \
         tc.tile_pool(name="p1work", bufs=2) as workp, \
         tc.tile_pool(name="p1g", bufs=12) as gp, \
         tc.tile_pool(name="p1s", bufs=8) as sp, \
         tc.tile_pool(name="p1ps", bufs=6, space="PSUM") as psp:

        idx_sb, coef, dloc_sb, denr = [], [], [], []
        with tc.tile_pool(name="p1prep", bufs=1) as prep:
            for r in range(R):
                t_idx = resp.tile([128, NCH * 8], I16, tag=f"idx{r}")
                nc.sync.dma_start(t_idx[:], srcidx[r][:])
                idx_sb.append(t_idx)

                t_dl = resp.tile([128, NCH], F32, tag=f"dl{r}")
                nc.sync.dma_start(t_dl[:], dstloc[r][:])
                dloc_sb.append(t_dl)

                t_dn = prep.tile([128, NCH], F32, tag="dn")
                nc.sync.dma_start(t_dn[:], dnum[r][:])
                t_dd = prep.tile([128, NCH], F32, tag="dd")
                nc.sync.dma_start(t_dd[:], dden[r][:])
                t_rd = prep.tile([128, NCH], F32, tag="rd")
                nc.vector.reciprocal(t_rd[:], t_dd[:])
                t_w = prep.tile([128, NCH], F32, tag="w")
                nc.vector.tensor_tensor(out=t_w[:], in0=t_dn[:], in1=t_rd[:], op=OP.mult)
                t_cf = resp.tile([128, NCH], F32, tag=f"cf{r}")
                nc.scalar.activation(t_cf[:], t_w[:], AF.Exp)
                coef.append(t_cf)

                # denominators: per-dst padded rows -> exp (bf16-rounded to
                # match the numerator S rounding) -> rowsum per tile
                t_cn = prep.tile([128, TILES * DMAX], F32, tag="cn")
                nc.sync.dma_start(t_cn[:], cpn[r][:])
                t_cd = prep.tile([128, TILES * DMAX], F32, tag="cd")
                nc.sync.dma_start(t_cd[:], cpd[r][:])
                t_crd = prep.tile([128, TILES * DMAX], F32, tag="crd")
                nc.vector.reciprocal(t_crd[:], t_cd[:])
                t_cw = prep.tile([128, TILES * DMAX], F32, tag="cw")
                nc.vector.tensor_tensor(out=t_cw[:], in0=t_cn[:], in1=t_crd[:], op=OP.mult)
                t_ce = prep.tile([128, TILES * DMAX], BF16, tag="ce")
                nc.scalar.activation(t_ce[:], t_cw[:], AF.Exp)
                t_den = prep.tile([128, TILES], F32, tag="den")
                nc.vector.reduce_sum(
                    t_den[:], t_ce[:].rearrange("p (t j) -> p t j", t=TILES),
                    axis=mybir.AxisListType.X)
                nc.vector.tensor_scalar(out=t_den[:], in0=t_den[:], scalar1=1e-9,
                                        scalar2=None, op0=OP.max)
                t_dr = resp.tile([128, TILES], F32, tag=f"dr{r}")
                nc.vector.reciprocal(t_dr[:], t_den[:])
                denr.append(t_dr)

        # piece layout: split each (tile, rel) gather into ring-sized pieces
        def mk_pieces(cpt_tr):
            pieces, p0 = [], 0
            while p0 < cpt_tr:
                pc = min(PIECE_CHUNKS, cpt_tr - p0)
                pieces.append((p0, pc))
                p0 += pc
            return pieces

        pid_sp = nc.sync.partition_id() if AG_MODE == "direct" else None
        G0 = None
        qctr = 0
        for t in range(TILES):
            nv = min(128, NSH - t * 128)
            hrow = workp.tile([128, HROW], BF16, tag="hrow")
            for r in range(R):
                cpt_tr = CPTS[t][r] if VAR_CPT else CPT
                if P1_MODE == "nogather":
                    if G0 is None:
                        G0 = resp.tile([128, CPT, D], BF16, tag="G0")
                        nc.gpsimd.dma_gather(
                            G0[:], xs[r][:], idx_sb[r][:, 0:CPT * 8],
                            CPT * 128, CPT * 128, D, single_packet=False)
                    Gs = [(G0, 0, cpt_tr)]
                else:
                    Gs = []
                    for (pstart, pc) in mk_pieces(cpt_tr):
                        G = gp.tile([128, PIECE_CHUNKS, D], BF16, tag="G")
                        c0 = t * CPT + pstart
                        nc.gpsimd.dma_gather(
                            G[:, :pc, :], xs[r][:],
                            idx_sb[r][:, c0 * 8:(c0 + pc) * 8],
                            pc * 128, pc * 128, D, single_packet=False,
                            queue_num=qctr % NQUEUES)
                        qctr += 1
                        Gs.append((G, pstart, pc))
                if P1_MODE == "gatheronly":
                    continue
                ps = psp.tile([128, D], F32, space="PSUM", tag="ps")
                for (G, pstart, pc) in Gs:
                    for jj in range(pc):
                        j = pstart + jj
                        g = t * CPT + j
                        S = sp.tile([128, 128], BF16, tag="S")
                        nc.vector.tensor_scalar(
                            out=S[:], in0=iota_b[:],
                            scalar1=dloc_sb[r][:, g:g + 1], scalar2=coef[r][:, g:g + 1],
                            op0=OP.is_equal, op1=OP.mult)
                        nc.tensor.matmul(ps[:], lhsT=S[:], rhs=G[:, jj, :],
                                         start=(j == 0), stop=(j == cpt_tr - 1))
                nc.vector.tensor_scalar(
                    out=hrow[:, r * D:(r + 1) * D], in0=ps[:],
                    scalar1=denr[r][:, t:t + 1], scalar2=None, op0=OP.mult)
            if P1_MODE == "gatheronly":
                continue
            nc.sync.dma_start(hsh[t * 128:t * 128 + nv, :], hrow[:nv, :])
            if AG_MODE == "direct":
                base = hfull[0:nv, :]
                wap = AP(base.tensor,
                         pid_sp * (NSH * HROW) + t * 128 * HROW, base.ap,
                         dep_tracking_offset=t * 128 * HROW)
                nc.sync.dma_start(wap, hrow[:nv, :])
            if AG_MODE == "collective" and AG_CHUNKS > 1:
                tpc = -(-TILES // AG_CHUNKS)              # tiles per chunk
                if (t + 1) % tpc == 0 or t == TILES - 1:
                    q = t // tpc
                    r0 = q * tpc * 128
                    r1 = min(NSH, (t + 1) * 128)
                    for _ag in range(AG_COUNT):
                        nc.gpsimd.collective_compute(
                            "AllGather", OP.bypass,
                            replica_groups=[list(range(NCORES))],
                            ins=[hsh[r0:r1, :]],
                            outs=[hfull[NCORES * r0:NCORES * r1, :]])

    if PHASES == "p1":
        return
    # ---------------- exchange ----------------
    if PHASES != "p2" and AG_MODE == "direct":
        # readback own hfull rows (orders after the direct writes), publish a
        # flag, barrier-AllGather it, and gate the Pool engine on completion.
        flagd = nc.dram_tensor("flagd", [1, TILES * 16], BF16)
        flagg = nc.dram_tensor("flagg", [NCORES, TILES * 16], BF16)
        with tc.tile_pool(name="barp", bufs=1) as barp:
            pid2 = nc.sync.partition_id()
            rb = barp.tile([1, TILES * 16], BF16)
            for t in range(TILES):
                rbase = hfull[0:1, 0:16]
                rap = AP(rbase.tensor,
                         pid2 * (NSH * HROW) + t * 128 * HROW, rbase.ap,
                         dep_tracking_offset=t * 128 * HROW)
                nc.sync.dma_start(rb[0:1, t * 16:(t + 1) * 16], rap)
            nc.sync.dma_start(flagd[0:1, :], rb[0:1, :])
            nc.gpsimd.collective_compute(
                "AllGather", OP.bypass, replica_groups=[list(range(NCORES))],
                ins=[flagd[:]], outs=[flagg[:]])
            junk = barp.tile([1, TILES * 16 * NCORES], BF16)
            nc.gpsimd.dma_start(junk[0:1, :], flagg[0:NCORES, :])
    if AG_MODE == "collective" and AG_CHUNKS == 1:
        for _ag in range(AG_COUNT):
            nc.gpsimd.collective_compute(
                "AllGather", OP.bypass, replica_groups=[list(range(NCORES))],
                ins=[hsh[:]], outs=[hfull[:]])

    # ---------------- phase 2 (two-stage software pipeline) ----------------
    with tc.tile_pool(name="p2res", bufs=1) as resp2, \
         tc.tile_pool(name="p2hc", bufs=3) as hcp, \
         tc.tile_pool(name="p2ht", bufs=3) as htp, \
         tc.tile_pool(name="p2mc", bufs=2) as mcp, \
         tc.tile_pool(name="p2sm", bufs=3) as smp:
        cidx = resp2.tile([128, TILES * K * 8], I16)
        nc.sync.dma_start(cidx[:], candidx[:])

        prev = None
        for t in range(TILES + 1):
            if t < TILES:
                # ---- stage A: gather, diff, squares, dist ----
                nvt = min(128, NSH - t * 128)
                Ht = htp.tile([128, HROW], BF16, tag="Ht")
                nc.sync.dma_start(Ht[:nvt, :], hsh[t * 128:t * 128 + nvt, :])
                Hc = hcp.tile([128, K, HROW], BF16, tag="Hc")
                if P2_SPLIT:
                    KH = K // 2
                    nc.gpsimd.dma_gather(
                        Hc[:, :KH, :], hfull[:], cidx[:, t * K * 8:t * K * 8 + KH * 8],
                        KH * 128, KH * 128, HROW, single_packet=False,
                        queue_num=(2 * t) % NQUEUES)
                    nc.gpsimd.dma_gather(
                        Hc[:, KH:, :], hfull[:], cidx[:, t * K * 8 + KH * 8:(t + 1) * K * 8],
                        KH * 128, KH * 128, HROW, single_packet=False,
                        queue_num=(2 * t + 1) % NQUEUES)
                else:
                    nc.gpsimd.dma_gather(
                        Hc[:], hfull[:], cidx[:, t * K * 8:(t + 1) * K * 8],
                        K * 128, K * 128, HROW, single_packet=False,
                        queue_num=t % NQUEUES)
                dist = smp.tile([128, K], F32, tag="dist")
                for k in range(K):
                    nc.vector.tensor_tensor(
                        out=Hc[:, k, :], in0=Ht[:], in1=Hc[:, k, :], op=OP.subtract)
                    nc.scalar.activation(Hc[:, k, :], Hc[:, k, :], AF.Square,
                                         accum_out=dist[:, k:k + 1])

            # ---- stage B for tile t-1: macc, mask, masked sum ----
            if prev is not None:
                pt, pHt, pHc, patt = prev
                nv = min(128, NSH - pt * 128)
                macc = mcp.tile([128, HROW], BF16, tag="macc")
                nc.vector.tensor_scalar(out=macc[:], in0=pHc[:, 0, :],
                                        scalar1=patt[:, 0:1], scalar2=None, op0=OP.mult)
                for k in range(1, K):
                    nc.vector.scalar_tensor_tensor(
                        out=macc[:], in0=pHc[:, k, :], scalar=patt[:, k:k + 1],
                        in1=macc[:], op0=OP.mult, op1=OP.add)
                mexp = mcp.tile([128, HROW], BF16, tag="mexp")
                nc.scalar.activation(mexp[:], macc[:], AF.Exp, scale=-1.0)
                h = mcp.tile([128, HROW], BF16, tag="h")
                nc.vector.tensor_tensor(out=h[:], in0=pHt[:], in1=mexp[:], op=OP.mult)
                a0 = smp.tile([128, D], BF16, tag="a0")
                nc.vector.tensor_tensor(out=a0[:], in0=h[:, 0:D], in1=h[:, D:2 * D], op=OP.add)
                a1 = smp.tile([128, D], BF16, tag="a1")
                nc.vector.tensor_tensor(out=a1[:], in0=h[:, 2 * D:3 * D], in1=h[:, 3 * D:4 * D], op=OP.add)
                osum = smp.tile([128, D], F32, tag="osum")
                nc.vector.tensor_tensor(out=osum[:], in0=a0[:], in1=a1[:], op=OP.add)
                nc.sync.dma_start(out[pt * 128:pt * 128 + nv, :], osum[:nv, :])

            if t < TILES:
                # ---- stage A cont.: att = softmax_k(-sqrt(dist)) ----
                s0 = smp.tile([128, K], F32, tag="s0")
                lg = smp.tile([128, K], F32, tag="lg")
                nc.scalar.activation(lg[:], dist[:], AF.Ln)
                nc.scalar.activation(s0[:], lg[:], AF.Exp, scale=0.5)
                rs0 = smp.tile([128, K], F32, tag="rs0")
                nc.vector.reciprocal(rs0[:], s0[:])
                rq = smp.tile([128, K], F32, tag="rq")
                nc.vector.tensor_tensor(out=rq[:], in0=dist[:], in1=rs0[:], op=OP.mult)
                nsd = smp.tile([128, K], F32, tag="nsd")
                nc.vector.tensor_tensor(out=nsd[:], in0=s0[:], in1=rq[:], op=OP.add)
                eu = smp.tile([128, K], F32, tag="eu")
                nc.scalar.activation(eu[:], nsd[:], AF.Exp, scale=-0.5)
                ssum = smp.tile([128, 1], F32, tag="ssum")
                nc.vector.reduce_sum(ssum[:], eu[:], axis=mybir.AxisListType.X)
                rs = smp.tile([128, 1], F32, tag="rs")
                nc.vector.reciprocal(rs[:], ssum[:])
                att = smp.tile([128, K], F32, tag="att")
                nc.vector.tensor_scalar(out=att[:], in0=eu[:], scalar1=rs[:, 0:1],
                                        scalar2=None, op0=OP.mult)
                prev = (t, Ht, Hc, att)


# ---------------------------------------------------------------- entry point
def kernel(x_src, d, d1, d2, src_idx, dst_idx, cand_idx, splitvulid):
    maps, CPT, DMAX, CPTS = _host_prep(x_src, d, d1, d2, src_idx, dst_idx,
                                       cand_idx, splitvulid)
    key = (CPT, DMAX, EMIT_REP, AG_COUNT, AG_CHUNKS, AG_SHARED, DMA_SCRATCH,
           PHASES, P1_MODE, NQUEUES, PIECE_CHUNKS, AG_MODE, VAR_CPT, CPTS,
           P2_SPLIT)
    if key not in _compiled:
        _compiled[key] = _build(CPT, DMAX, CPTS)
    nc = _compiled[key]
    res = run_bass_kernel_spmd(nc, maps, list(range(NCORES)))
    return np.concatenate([res.results[c]["out"] for c in range(NCORES)], axis=0)


# revision 24
# speedup vs baseline: 1.5496x; 1.5496x over previous
"""Trainium2 Bass kernel for nn_AggregateLayer (gnn_message_passing).

Strategy (8 NeuronCores, dst-node sharding):
  - Host: route/sort edges by (core, dst-tile), pad to uniform chunk counts,
    build int16 gather-index wraps and per-edge scalar arrays (pure layout).
    x is pre-cast to bf16 (halves the edge-gather HBM traffic).
  - Phase 1 (per core, 2500 dst nodes): per relation, dma_gather bf16 x rows
    for each 128-edge chunk, build the bf16 scatter matrix
    S[e, dstlocal] = coef_e via iota/is_equal/mult on DVE, and accumulate
    PSUM[dst, :] += S^T @ G on the PE.  Denominators via per-dst padded
    coefficient rows + free-dim reduce (bf16-rounded exp to match the
    numerator's rounding).
  - AllGather bf16 H shards (5MB/core) into a Shared DRAM tensor, split in
    row chunks overlapped with phase 1.
  - Phase 2: two-stage software pipeline per 128-node tile: stage A gathers
    the K=16 candidate bf16 H rows, computes diff/squares/dist and the
    softmax att; stage B (emitted one tile later) does the att-weighted
    squared-diff mask and the final masked relation sum.  All bulk ops in
    bf16 for 2x DVE throughput.
"""

import numpy as np
import ml_dtypes

import concourse.bacc as bacc
from concourse.bass import AP
import concourse.mybir as mybir
import concourse.tile as tile
from concourse.bass_utils import run_bass_kernel_spmd
from concourse.library_config import mlp
from bass_rust import InstNoOp

F32 = mybir.dt.float32
BF16 = mybir.dt.bfloat16
I16 = mybir.dt.int16
AF = mybir.ActivationFunctionType
OP = mybir.AluOpType

R, NSRC, NVUL, D, E, K = 4, 20000, 20000, 256, 640000, 16
NCORES = 8
NSH = NVUL // NCORES          # 2500 dst nodes per core
TILES = (NSH + 127) // 128    # 20 tiles (last has 68 valid rows)
HROW = R * D                  # 1024 els per H row

# knobs
AG_COUNT = 1                  # timing instrument: emit AllGather this many times
AG_CHUNKS = 7                 # >1: split AllGather into row-chunks overlapped with phase 1
AG_SHARED = True              # AllGather into a Shared DRAM tensor (single copy)
DMA_SCRATCH = 32768           # SWDGE ring bytes
EMIT_REP = 1                  # repeat whole compute pass (timing instrument)
PHASES = "both"               # timing instrument: "both" | "p1" (skip phase2)
P1_MODE = "full"              # timing instrument: "full" | "nogather" | "gatheronly"
NQUEUES = 4                   # SWDGE queues; desc-gen parallelism across Q7 pairs
PIECE_CHUNKS = 6              # P1 gather piece size (chunks); ring holds 2048 descs
AG_MODE = "collective"        # "collective" payload AllGather | "direct" writes+barrier
VAR_CPT = True                # per-(tile,rel) chunk counts (max over cores) baked at compile
P2_SPLIT = True               # split phase-2 gather into two queue-spread pieces
S_BUFS = 16                   # S-matrix ring depth (DVE->PE pipelining)
PS_BUFS = 8                   # PSUM ring depth
HC_BUFS = 4                   # phase-2 candidate tile ring depth

_compiled = {}


# ---------------------------------------------------------------- host prep
def _wrap16(a):
    """dma_gather index layout: element i -> [i % 16, i // 16], tiled to 128
    partitions (8 Q7-core replicas)."""
    a = np.asarray(a, np.int16)
    pad = (-len(a)) % 16
    if pad:
        a = np.concatenate([a, np.zeros(pad, np.int16)])
    m = a.reshape(-1, 16).T
    return np.tile(m, (8, 1))


def _chunkify(v, cpt, fill):
    """[20, cpt*128] padded per-tile edge values -> [128, 20*cpt] chunk-major
    layout (edge t*cpt*128 + j*128 + p -> [p, t*cpt + j])."""
    out = v.reshape(TILES, cpt, 128).transpose(2, 0, 1).reshape(128, TILES * cpt)
    return np.ascontiguousarray(out)


def _host_prep(x_src, d, d1, d2, src_idx, dst_idx, cand_idx, splitvulid):
    split = int(splitvulid)
    x_src = np.asarray(x_src, np.float32)
    d = np.asarray(d, np.float32)
    d1 = np.asarray(d1, np.float32)
    d2 = np.asarray(d2, np.float32)
    src_idx = np.asarray(src_idx)
    dst_idx = np.asarray(dst_idx)
    cand_idx = np.asarray(cand_idx)

    # sort each relation's edges by dst once; split per core by searchsorted
    per_r = []
    for r in range(R):
        order = np.argsort(dst_idx[r], kind="stable")
        ds = dst_idx[r][order]
        ss = src_idx[r][order]
        bounds = np.searchsorted(ds, np.arange(0, NVUL + 1, NSH))
        per_r.append((ds, ss, bounds))

    # global uniform chunk count per dst-tile and max degree
    max_tile_edges = 0
    max_deg = 0
    tile_max = np.zeros((R, TILES), np.int64)   # per-(rel,tile) max edges over cores
    for r in range(R):
        ds, ss, bounds = per_r[r]
        for c in range(NCORES):
            dloc = ds[bounds[c]:bounds[c + 1]] - c * NSH
            tc_counts = np.bincount(dloc // 128, minlength=TILES)
            np.maximum(tile_max[r], tc_counts, out=tile_max[r])
            max_tile_edges = max(max_tile_edges, int(tc_counts.max()))
            deg = np.bincount(dloc, minlength=NSH)
            max_deg = max(max_deg, int(deg.max()))
    CPT = -(-max_tile_edges // 128)          # chunks per dst tile
    CPT += -CPT % 2                          # round to even (compile-cache)
    DMAX = max_deg + (-max_deg % 8)
    NCH = TILES * CPT
    CPTS = tuple(tuple(max(1, -(-int(tile_max[r][t]) // 128)) for r in range(R))
                 for t in range(TILES))

    maps = []
    for c in range(NCORES):
        m = {}
        for r in range(R):
            ds, ss, bounds = per_r[r]
            sl = slice(bounds[c], bounds[c + 1])
            dloc = ds[sl] - c * NSH
            sloc = ss[sl]
            dglob = ds[sl]
            nume = len(dloc)

            # per-edge scalars: dnum = d1[src] (dst<split) else -d2[src]
            use1 = dglob < split
            dnum = np.where(use1, d1[r][sloc], -d2[r][sloc]).astype(np.float32)
            dden = d[r][sloc].astype(np.float32)

            # scatter edges into per-tile padded slots [20, CPT*128]
            tid = dloc // 128
            starts = np.zeros(TILES, np.int64)
            cnt = np.bincount(tid, minlength=TILES)
            starts[1:] = np.cumsum(cnt)[:-1]
            pos = np.arange(nume) - starts[tid]     # position within tile
            slot = tid * (CPT * 128) + pos

            src_pad = np.zeros(TILES * CPT * 128, np.int16)
            dl_pad = np.full(TILES * CPT * 128, 200.0, np.float32)
            dn_pad = np.full(TILES * CPT * 128, -1e30, np.float32)
            dd_pad = np.ones(TILES * CPT * 128, np.float32)
            src_pad[slot] = sloc.astype(np.int16)
            dl_pad[slot] = (dloc % 128).astype(np.float32)
            dn_pad[slot] = dnum
            dd_pad[slot] = dden

            m[f"srcidx{r}"] = _wrap16(src_pad)
            m[f"dstloc{r}"] = _chunkify(dl_pad, CPT, 200.0)
            m[f"dnum{r}"] = _chunkify(dn_pad, CPT, -1e30)
            m[f"dden{r}"] = _chunkify(dd_pad, CPT, 1.0)

            # per-dst padded coefficient rows for the denominators
            deg = np.bincount(dloc, minlength=NSH)
            dstart = np.zeros(NSH, np.int64)
            dstart[1:] = np.cumsum(deg)[:-1]
            dpos = np.arange(nume) - dstart[dloc]
            cn = np.full((TILES * 128, DMAX), -1e30, np.float32)
            cd = np.ones((TILES * 128, DMAX), np.float32)
            cn[dloc, dpos] = dnum
            cd[dloc, dpos] = dden
            m[f"cpn{r}"] = np.ascontiguousarray(
                cn.reshape(TILES, 128, DMAX).transpose(1, 0, 2).reshape(128, TILES * DMAX))
            m[f"cpd{r}"] = np.ascontiguousarray(
                cd.reshape(TILES, 128, DMAX).transpose(1, 0, 2).reshape(128, TILES * DMAX))
            m[f"x{r}"] = np.ascontiguousarray(x_src[r].astype(ml_dtypes.bfloat16))

        # phase-2 candidate indices, per tile wrap (remapped to the chunked
        # hfull layout when the exchange is split into row-chunk AllGathers)
        if AG_CHUNKS > 1 and AG_MODE == "collective":
            rows_per = -(-TILES // AG_CHUNKS) * 128        # rows per chunk (tile-aligned)
            def remap(n):
                cc, loc = n // NSH, n % NSH
                q = np.minimum(loc // rows_per, AG_CHUNKS - 1)
                sz = np.minimum(NSH - q * rows_per, rows_per)
                base = NCORES * rows_per * q
                return base + cc * sz + (loc - q * rows_per)
        else:
            remap = lambda n: n
        ci = np.zeros((TILES, K * 128), np.int64)
        for t in range(TILES):
            base = c * NSH + t * 128
            nv = min(128, NSH - t * 128)
            blk = np.zeros((K, 128), np.int64)
            blk[:, :nv] = remap(cand_idx[base:base + nv, :].astype(np.int64)).T
            ci[t] = blk.reshape(-1)
        wr = np.concatenate([_wrap16(ci[t]) for t in range(TILES)], axis=1)
        m["candidx"] = wr
        maps.append(m)
    return maps, CPT, DMAX, CPTS


# ---------------------------------------------------------------- device build
def _fix_multiwaits(nc, limit=1):
    """This walrus build rejects >1-2 sem waits on one instruction; hoist
    excess waits onto same-engine NOPs inserted just before."""
    ctr = 0
    for bb in nc.m.functions[0].blocks:
        insts = bb.instructions
        out = []
        for inst in insts:
            si = inst.sync_info
            waits = list(si.on_wait) if (si and si.on_wait) else []
            if len(waits) > limit:
                excess, keep = waits[:-limit], waits[-limit:]
                for i in range(0, len(excess), limit):
                    ctr += 1
                    n = InstNoOp(name=f"I-mwfix-{ctr}", hint="mwfix")
                    n.engine = inst.engine
                    n.sync_info = mybir.SyncInfo(
                        on_wait=excess[i:i + limit], on_update=[])
                    out.append(n)
                si.on_wait = keep
            out.append(inst)
        if len(out) != len(insts):
            insts[:] = out
    return nc


def _build(CPT, DMAX, CPTS):
    NCH = TILES * CPT
    nc = bacc.Bacc("TRN2", target_bir_lowering=False, debug=False,
                   dynamic_dma_scratch_size=DMA_SCRATCH,
                   num_swdge_queues=NQUEUES)

    xs = [nc.declare_dram_parameter(f"x{r}", [NSRC, D], BF16, isOutput=False)
          for r in range(R)]
    srcidx = [nc.declare_dram_parameter(f"srcidx{r}", [128, NCH * 8], I16, isOutput=False)
              for r in range(R)]
    dstloc = [nc.declare_dram_parameter(f"dstloc{r}", [128, NCH], F32, isOutput=False)
              for r in range(R)]
    dnum = [nc.declare_dram_parameter(f"dnum{r}", [128, NCH], F32, isOutput=False)
            for r in range(R)]
    dden = [nc.declare_dram_parameter(f"dden{r}", [128, NCH], F32, isOutput=False)
            for r in range(R)]
    cpn = [nc.declare_dram_parameter(f"cpn{r}", [128, TILES * DMAX], F32, isOutput=False)
           for r in range(R)]
    cpd = [nc.declare_dram_parameter(f"cpd{r}", [128, TILES * DMAX], F32, isOutput=False)
           for r in range(R)]
    candidx = nc.declare_dram_parameter("candidx", [128, TILES * K * 8], I16, isOutput=False)
    out = nc.declare_dram_parameter("out", [NSH, D], F32, isOutput=True)

    hsh = nc.dram_tensor("hsh", [NSH, HROW], BF16)
    hfull = nc.dram_tensor("hfull", [NVUL, HROW], BF16,
                           addr_space="Shared" if AG_SHARED else "Local")

    with tile.TileContext(nc) as tc:
        with tc.tile_pool(name="const", bufs=1) as constp:
            nc.gpsimd.load_library(mlp)
            iota_i = constp.tile([128, 128], mybir.dt.int32)
            nc.gpsimd.iota(iota_i[:], pattern=[[1, 128]], base=0, channel_multiplier=0)
            iota_f = constp.tile([128, 128], F32)
            nc.vector.tensor_copy(iota_f[:], iota_i[:])
            iota_b = constp.tile([128, 128], BF16)
            nc.vector.tensor_copy(iota_b[:], iota_f[:])

            for rep in range(EMIT_REP):
                _emit_pass(nc, tc, iota_b, xs, srcidx, dstloc, dnum, dden,
                           cpn, cpd, candidx, out, hsh, hfull, CPT, DMAX, CPTS)

    _fix_multiwaits(nc)
    nc.compile()
    return nc


def _emit_pass(nc, tc, iota_b, xs, srcidx, dstloc, dnum, dden, cpn, cpd,
               candidx, out, hsh, hfull, CPT, DMAX, CPTS):
    NCH = TILES * CPT

    # ---------------- phase 1 ----------------
    if PHASES != "p2":
      with tc.tile_pool(name="p1res", bufs=1) as resp, \
         tc.tile_pool(name="p1work", bufs=2) as workp, \
         tc.tile_pool(name="p1g", bufs=12) as gp, \
         tc.tile_pool(name="p1s", bufs=S_BUFS) as sp, \
         tc.tile_pool(name="p1ps", bufs=PS_BUFS, space="PSUM") as psp:

        idx_sb, coef, dloc_sb, denr = [], [], [], []
        with tc.tile_pool(name="p1prep", bufs=1) as prep:
            for r in range(R):
                t_idx = resp.tile([128, NCH * 8], I16, tag=f"idx{r}")
                nc.sync.dma_start(t_idx[:], srcidx[r][:])
                idx_sb.append(t_idx)

                t_dl = resp.tile([128, NCH], F32, tag=f"dl{r}")
                nc.sync.dma_start(t_dl[:], dstloc[r][:])
                dloc_sb.append(t_dl)

                t_dn = prep.tile([128, NCH], F32, tag="dn")
                nc.sync.dma_start(t_dn[:], dnum[r][:])
                t_dd = prep.tile([128, NCH], F32, tag="dd")
                nc.sync.dma_start(t_dd[:], dden[r][:])
                t_rd = prep.tile([128, NCH], F32, tag="rd")
                nc.vector.reciprocal(t_rd[:], t_dd[:])
                t_w = prep.tile([128, NCH], F32, tag="w")
                nc.vector.tensor_tensor(out=t_w[:], in0=t_dn[:], in1=t_rd[:], op=OP.mult)
                t_cf = resp.tile([128, NCH], F32, tag=f"cf{r}")
                nc.scalar.activation(t_cf[:], t_w[:], AF.Exp)
                coef.append(t_cf)

                # denominators: per-dst padded rows -> exp (bf16-rounded to
                # match the numerator S rounding) -> rowsum per tile
                t_cn = prep.tile([128, TILES * DMAX], F32, tag="cn")
                nc.sync.dma_start(t_cn[:], cpn[r][:])
                t_cd = prep.tile([128, TILES * DMAX], F32, tag="cd")
                nc.sync.dma_start(t_cd[:], cpd[r][:])
                t_crd = prep.tile([128, TILES * DMAX], F32, tag="crd")
                nc.vector.reciprocal(t_crd[:], t_cd[:])
                t_cw = prep.tile([128, TILES * DMAX], F32, tag="cw")
                nc.vector.tensor_tensor(out=t_cw[:], in0=t_cn[:], in1=t_crd[:], op=OP.mult)
                t_ce = prep.tile([128, TILES * DMAX], BF16, tag="ce")
                nc.scalar.activation(t_ce[:], t_cw[:], AF.Exp)
                t_den = prep.tile([128, TILES], F32, tag="den")
                nc.vector.reduce_sum(
                    t_den[:], t_ce[:].rearrange("p (t j) -> p t j", t=TILES),
                    axis=mybir.AxisListType.X)
                nc.vector.tensor_scalar(out=t_den[:], in0=t_den[:], scalar1=1e-9,
                                        scalar2=None, op0=OP.max)
                t_dr = resp.tile([128, TILES], F32, tag=f"dr{r}")
                nc.vector.reciprocal(t_dr[:], t_den[:])
                denr.append(t_dr)

        # piece layout: split each (tile, rel) gather into ring-sized pieces
        def mk_pieces(cpt_tr):
            pieces, p0 = [], 0
            while p0 < cpt_tr:
                pc = min(PIECE_CHUNKS, cpt_tr - p0)
                pieces.append((p0, pc))
                p0 += pc
            return pieces

        pid_sp = nc.sync.partition_id() if AG_MODE == "direct" else None
        G0 = None
        qctr = 0
        for t in range(TILES):
            nv = min(128, NSH - t * 128)
            hrow = workp.tile([128, HROW], BF16, tag="hrow")
            for r in range(R):
                cpt_tr = CPTS[t][r] if VAR_CPT else CPT
                if P1_MODE == "nogather":
                    if G0 is None:
                        G0 = resp.tile([128, CPT, D], BF16, tag="G0")
                        nc.gpsimd.dma_gather(
                            G0[:], xs[r][:], idx_sb[r][:, 0:CPT * 8],
                            CPT * 128, CPT * 128, D, single_packet=False)
                    Gs = [(G0, 0, cpt_tr)]
                else:
                    Gs = []
                    for (pstart, pc) in mk_pieces(cpt_tr):
                        G = gp.tile([128, PIECE_CHUNKS, D], BF16, tag="G")
                        c0 = t * CPT + pstart
                        nc.gpsimd.dma_gather(
                            G[:, :pc, :], xs[r][:],
                            idx_sb[r][:, c0 * 8:(c0 + pc) * 8],
                            pc * 128, pc * 128, D, single_packet=False,
                            queue_num=qctr % NQUEUES)
                        qctr += 1
                        Gs.append((G, pstart, pc))
                if P1_MODE == "gatheronly":
                    continue
                ps = psp.tile([128, D], F32, space="PSUM", tag="ps")
                for (G, pstart, pc) in Gs:
                    for jj in range(pc):
                        j = pstart + jj
                        g = t * CPT + j
                        S = sp.tile([128, 128], BF16, tag="S")
                        nc.vector.tensor_scalar(
                            out=S[:], in0=iota_b[:],
                            scalar1=dloc_sb[r][:, g:g + 1], scalar2=coef[r][:, g:g + 1],
                            op0=OP.is_equal, op1=OP.mult)
                        nc.tensor.matmul(ps[:], lhsT=S[:], rhs=G[:, jj, :],
                                         start=(j == 0), stop=(j == cpt_tr - 1))
                nc.vector.tensor_scalar(
                    out=hrow[:, r * D:(r + 1) * D], in0=ps[:],
                    scalar1=denr[r][:, t:t + 1], scalar2=None, op0=OP.mult)
            if P1_MODE == "gatheronly":
                continue
            nc.sync.dma_start(hsh[t * 128:t * 128 + nv, :], hrow[:nv, :])
            if AG_MODE == "direct":
                base = hfull[0:nv, :]
                wap = AP(base.tensor,
                         pid_sp * (NSH * HROW) + t * 128 * HROW, base.ap,
                         dep_tracking_offset=t * 128 * HROW)
                nc.sync.dma_start(wap, hrow[:nv, :])
            if AG_MODE == "collective" and AG_CHUNKS > 1:
                tpc = -(-TILES // AG_CHUNKS)              # tiles per chunk
                if (t + 1) % tpc == 0 or t == TILES - 1:
                    q = t // tpc
                    r0 = q * tpc * 128
                    r1 = min(NSH, (t + 1) * 128)
                    for _ag in range(AG_COUNT):
                        nc.gpsimd.collective_compute(
                            "AllGather", OP.bypass,
                            replica_groups=[list(range(NCORES))],
                            ins=[hsh[r0:r1, :]],
                            outs=[hfull[NCORES * r0:NCORES * r1, :]])

    if PHASES == "p1":
        return
    # ---------------- exchange ----------------
    if PHASES != "p2" and AG_MODE == "direct":
        # readback own hfull rows (orders after the direct writes), publish a
        # flag, barrier-AllGather it, and gate the Pool engine on completion.
        flagd = nc.dram_tensor("flagd", [1, TILES * 16], BF16)
        flagg = nc.dram_tensor("flagg", [NCORES, TILES * 16], BF16)
        with tc.tile_pool(name="barp", bufs=1) as barp:
            pid2 = nc.sync.partition_id()
            rb = barp.tile([1, TILES * 16], BF16)
            for t in range(TILES):
                rbase = hfull[0:1, 0:16]
                rap = AP(rbase.tensor,
                         pid2 * (NSH * HROW) + t * 128 * HROW, rbase.ap,
                         dep_tracking_offset=t * 128 * HROW)
                nc.sync.dma_start(rb[0:1, t * 16:(t + 1) * 16], rap)
            nc.sync.dma_start(flagd[0:1, :], rb[0:1, :])
            nc.gpsimd.collective_compute(
                "AllGather", OP.bypass, replica_groups=[list(range(NCORES))],
                ins=[flagd[:]], outs=[flagg[:]])
            junk = barp.tile([1, TILES * 16 * NCORES], BF16)
            nc.gpsimd.dma_start(junk[0:1, :], flagg[0:NCORES, :])
    if AG_MODE == "collective" and AG_CHUNKS == 1:
        for _ag in range(AG_COUNT):
            nc.gpsimd.collective_compute(
                "AllGather", OP.bypass, replica_groups=[list(range(NCORES))],
                ins=[hsh[:]], outs=[hfull[:]])

    # ---------------- phase 2 (two-stage software pipeline) ----------------
    with tc.tile_pool(name="p2res", bufs=1) as resp2, \
         tc.tile_pool(name="p2hc", bufs=HC_BUFS) as hcp, \
         tc.tile_pool(name="p2ht", bufs=3) as htp, \
         tc.tile_pool(name="p2mc", bufs=2) as mcp, \
         tc.tile_pool(name="p2sm", bufs=3) as smp:
        cidx = resp2.tile([128, TILES * K * 8], I16)
        nc.sync.dma_start(cidx[:], candidx[:])

        prev = None
        for t in range(TILES + 1):
            if t < TILES:
                # ---- stage A: gather, diff, squares, dist ----
                nvt = min(128, NSH - t * 128)
                Ht = htp.tile([128, HROW], BF16, tag="Ht")
                nc.sync.dma_start(Ht[:nvt, :], hsh[t * 128:t * 128 + nvt, :])
                Hc = hcp.tile([128, K, HROW], BF16, tag="Hc")
                if P2_SPLIT:
                    KH = K // 2
                    nc.gpsimd.dma_gather(
                        Hc[:, :KH, :], hfull[:], cidx[:, t * K * 8:t * K * 8 + KH * 8],
                        KH * 128, KH * 128, HROW, single_packet=False,
                        queue_num=(2 * t) % NQUEUES)
                    nc.gpsimd.dma_gather(
                        Hc[:, KH:, :], hfull[:], cidx[:, t * K * 8 + KH * 8:(t + 1) * K * 8],
                        KH * 128, KH * 128, HROW, single_packet=False,
                        queue_num=(2 * t + 1) % NQUEUES)
                else:
                    nc.gpsimd.dma_gather(
                        Hc[:], hfull[:], cidx[:, t * K * 8:(t + 1) * K * 8],
                        K * 128, K * 128, HROW, single_packet=False,
                        queue_num=t % NQUEUES)
                dist = smp.tile([128, K], F32, tag="dist")
                for k in range(K):
                    nc.vector.tensor_tensor(
                        out=Hc[:, k, :], in0=Ht[:], in1=Hc[:, k, :], op=OP.subtract)
                    nc.scalar.activation(Hc[:, k, :], Hc[:, k, :], AF.Square,
                                         accum_out=dist[:, k:k + 1])

            # ---- stage B for tile t-1: macc, mask, masked sum ----
            if prev is not None:
                pt, pHt, pHc, patt = prev
                nv = min(128, NSH - pt * 128)
                macc = mcp.tile([128, HROW], BF16, tag="macc")
                nc.vector.tensor_scalar(out=macc[:], in0=pHc[:, 0, :],
                                        scalar1=patt[:, 0:1], scalar2=None, op0=OP.mult)
                for k in range(1, K):
                    nc.vector.scalar_tensor_tensor(
                        out=macc[:], in0=pHc[:, k, :], scalar=patt[:, k:k + 1],
                        in1=macc[:], op0=OP.mult, op1=OP.add)
                mexp = mcp.tile([128, HROW], BF16, tag="mexp")
                nc.scalar.activation(mexp[:], macc[:], AF.Exp, scale=-1.0)
                h = mcp.tile([128, HROW], BF16, tag="h")
                nc.vector.tensor_tensor(out=h[:], in0=pHt[:], in1=mexp[:], op=OP.mult)
                a0 = smp.tile([128, D], BF16, tag="a0")
                nc.vector.tensor_tensor(out=a0[:], in0=h[:, 0:D], in1=h[:, D:2 * D], op=OP.add)
                a1 = smp.tile([128, D], BF16, tag="a1")
                nc.vector.tensor_tensor(out=a1[:], in0=h[:, 2 * D:3 * D], in1=h[:, 3 * D:4 * D], op=OP.add)
                osum = smp.tile([128, D], F32, tag="osum")
                nc.vector.tensor_tensor(out=osum[:], in0=a0[:], in1=a1[:], op=OP.add)
                nc.sync.dma_start(out[pt * 128:pt * 128 + nv, :], osum[:nv, :])

            if t < TILES:
                # ---- stage A cont.: att = softmax_k(-sqrt(dist)) ----
                s0 = smp.tile([128, K], F32, tag="s0")
                lg = smp.tile([128, K], F32, tag="lg")
                nc.scalar.activation(lg[:], dist[:], AF.Ln)
                nc.scalar.activation(s0[:], lg[:], AF.Exp, scale=0.5)
                rs0 = smp.tile([128, K], F32, tag="rs0")
                nc.vector.reciprocal(rs0[:], s0[:])
                rq = smp.tile([128, K], F32, tag="rq")
                nc.vector.tensor_tensor(out=rq[:], in0=dist[:], in1=rs0[:], op=OP.mult)
                nsd = smp.tile([128, K], F32, tag="nsd")
                nc.vector.tensor_tensor(out=nsd[:], in0=s0[:], in1=rq[:], op=OP.add)
                eu = smp.tile([128, K], F32, tag="eu")
                nc.scalar.activation(eu[:], nsd[:], AF.Exp, scale=-0.5)
                ssum = smp.tile([128, 1], F32, tag="ssum")
                nc.vector.reduce_sum(ssum[:], eu[:], axis=mybir.AxisListType.X)
                rs = smp.tile([128, 1], F32, tag="rs")
                nc.vector.reciprocal(rs[:], ssum[:])
                att = smp.tile([128, K], F32, tag="att")
                nc.vector.tensor_scalar(out=att[:], in0=eu[:], scalar1=rs[:, 0:1],
                                        scalar2=None, op0=OP.mult)
                prev = (t, Ht, Hc, att)


# ---------------------------------------------------------------- entry point
def kernel(x_src, d, d1, d2, src_idx, dst_idx, cand_idx, splitvulid):
    maps, CPT, DMAX, CPTS = _host_prep(x_src, d, d1, d2, src_idx, dst_idx,
                                       cand_idx, splitvulid)
    key = (CPT, DMAX, EMIT_REP, AG_COUNT, AG_CHUNKS, AG_SHARED, DMA_SCRATCH,
           PHASES, P1_MODE, NQUEUES, PIECE_CHUNKS, AG_MODE, VAR_CPT, CPTS,
           P2_SPLIT, S_BUFS, PS_BUFS, HC_BUFS)
    if key not in _compiled:
        _compiled[key] = _build(CPT, DMAX, CPTS)
    nc = _compiled[key]
    res = run_bass_kernel_spmd(nc, maps, list(range(NCORES)))
    return np.concatenate([res.results[c]["out"] for c in range(NCORES)], axis=0)
